# revision 24
# baseline (speedup 1.0000x reference)
"""Multi-head attention (B=4, T=2048, D=512, 8 heads) on 8 Trainium2 cores.

Sharding: core = (batch b, head-group g) with 4 heads per group —
8 cores = 4 batches x 2 groups.  Each core computes its group's
projections, scores+softmax (attn output), attn@V and the partial
out-projection; the host sums the two group-partials per batch and
adds the biases that commute through softmax (b_v, b_o).

Projections run as float32r (tf32-like, full PE rate); the attention
inner loop uses fp16 (10-bit mantissa, same precision class as f32r):
fp16 enables the DMA-xbar transpose for the attn^T tiles that attn@V
needs (PE transposes of 4-byte data are LDWEIGHTS-bound and slow) and
a casting SWDGE DMA writes the fp32 attn output straight from fp16.
Softmax itself is fp32 on the Scalar engine with a fused row-sum.
"""
import numpy as np
import concourse.bass as bass
import concourse.tile as tile
from concourse import mybir
from concourse.bass_utils import run_bass_kernel_spmd

P = 128
T = 2048
D = 512
NH = 8
DH = 64
GH = 4           # heads per group (per core)
GW = GH * DH     # 256 group width
NT = T // P      # 16 seq tiles
F32 = mybir.dt.float32
F32R = mybir.dt.float32r
F16 = mybir.dt.float16
EXP = mybir.ActivationFunctionType.Exp
IDENT = mybir.ActivationFunctionType.Identity


# --- walrus in this container rejects >1 sync wait per instruction; move the
# extras onto same-engine NoOps inserted just before (same semantics). ---
_patched = False


def _install_wait_split():
    global _patched
    if _patched:
        return
    _patched = True
    from concourse import bass2jax, bass_utils

    orig = bass_utils.compile_bir_kernel

    def strip_xbar(bir):
        # Tile mutually serializes DMA transposes against all other DMAs
        # (conservative guard for the transpose-vs-SBUF2SBUF HW deadlock).
        # This kernel has no SBUF->SBUF copies: the only DMASW traffic is the
        # attn output store and the only transposes write expT, with no data
        # overlap — so the cross waits are pure serialization. Drop them.
        n = 0
        for fn in bir.get("functions", []):
            for blk in fn.get("blocks", []):
                for inst in blk.get("instructions", []):
                    si = inst.get("sync_info")
                    waits = (si or {}).get("on_wait") or []
                    if not waits:
                        continue
                    if inst["opcode"] == "DmaTransposeAnt":
                        drop = [w for w in waits
                                if str(w.get("ant_name", "")).startswith("DMASW")]
                    elif (inst["opcode"] == "DMACopy"
                          and inst.get("engine") == "Pool"):
                        drop = [w for w in waits
                                if str(w.get("ant_name", "")).startswith("DMAHW")]
                    else:
                        continue
                    if drop:
                        si["on_wait"] = [w for w in waits if w not in drop]
                        n += len(drop)
        return n

    def split(bir):
        n = 0
        for fn in bir.get("functions", []):
            for blk in fn.get("blocks", []):
                out = []
                for inst in blk.get("instructions", []):
                    si = inst.get("sync_info")
                    waits = (si or {}).get("on_wait") or []
                    if len(waits) > 1:
                        for k, w in enumerate(waits[:-1]):
                            out.append({
                                "debug": inst.get("debug", 0),
                                "engine": inst["engine"],
                                "ins": [], "outs": [],
                                "name": f"{inst['name']}_wsplit{k}",
                                "opcode": "NoOp",
                                "text_hint": "waitsplit",
                                "sync_info": {"on_wait": [w], "on_update": []},
                            })
                            n += 1
                        si["on_wait"] = [waits[-1]]
                    out.append(inst)
                blk["instructions"] = out
        return n

    def patched(bir_json, tmpdir, neff_name="file.neff"):
        import json as _j
        bir = _j.loads(bir_json)
        changed = strip_xbar(bir)
        changed += split(bir)
        if changed:
            bir_json = _j.dumps(bir).encode()
        return orig(bir_json, tmpdir, neff_name)

    bass_utils.compile_bir_kernel = patched
    bass2jax.compile_bir_kernel = patched


def _build_program():
    nc = bass.Bass()

    qt_d = nc.dram_tensor("qt", [D, T], F32, kind="ExternalInput")
    kt_d = nc.dram_tensor("kt", [D, T], F32, kind="ExternalInput")
    vt_d = nc.dram_tensor("vt", [D, T], F32, kind="ExternalInput")
    wq_d = nc.dram_tensor("wq", [D, GW], F32, kind="ExternalInput")
    wk_d = nc.dram_tensor("wk", [D, GW], F32, kind="ExternalInput")
    wv_d = nc.dram_tensor("wv", [D, GW], F32, kind="ExternalInput")
    wo_d = nc.dram_tensor("wo", [GW, D], F32, kind="ExternalInput")
    bq_d = nc.dram_tensor("bq", [P, 2], F32, kind="ExternalInput")
    bk_d = nc.dram_tensor("bk", [P, 2], F32, kind="ExternalInput")
    eye_d = nc.dram_tensor("eye", [P, 2 * P], F32, kind="ExternalInput")
    attn_d = nc.dram_tensor("attn_out", [GH, T, T], F32, kind="ExternalOutput")
    out_d = nc.dram_tensor("out_part", [T, D], F32, kind="ExternalOutput")

    with tile.TileContext(nc) as tc:
        with tc.tile_pool(name="const", bufs=1) as const, \
             tc.tile_pool(name="qkv", bufs=1) as qkv, \
             tc.tile_pool(name="scores", bufs=2, space="PSUM") as scores_ps, \
             tc.tile_pool(name="trg", bufs=1, space="PSUM") as trg_ps, \
             tc.tile_pool(name="ps512", bufs=2, space="PSUM") as ps512:

            # ---------- constants ----------
            bq = const.tile([P, 2], F32, tag="bq")
            bk = const.tile([P, 2], F32, tag="bk")
            nc.sync.dma_start(bq[:], bq_d[:])
            nc.sync.dma_start(bk[:], bk_d[:])
            eye_f = const.tile([P, 2 * P], F32, tag="eyef")
            nc.sync.dma_start(eye_f[:], eye_d[:])
            eye16 = const.tile([P, 2 * P], F16, tag="eye16")
            nc.vector.tensor_copy(eye16[:], eye_f[:])

            w_r = {}
            for nm, dram in (("wq", wq_d), ("wk", wk_d), ("wv", wv_d)):
                wf = const.tile([P, 4, GW], F32, tag="wstage")
                nc.sync.dma_start(wf[:], dram.rearrange("(ko p) m -> p ko m", p=P))
                wr = const.tile([P, 4, GW], F32R, tag=f"{nm}r")
                nc.vector.tensor_copy(wr[:], wf[:])
                w_r[nm] = wr
            wof = const.tile([P, 2, D], F32, tag="wof")
            nc.sync.dma_start(wof[:], wo_d.rearrange("(pp p) m -> p pp m", p=P))
            wo = const.tile([P, 2, D], F32R, tag="wo")
            nc.vector.tensor_copy(wo[:], wof[:])

            # ---------- persistent activations ----------
            qT = qkv.tile([P, 2, T], F16, tag="qT")    # [dh-of-pair, pair, seq]
            kT = qkv.tile([P, 2, T], F16, tag="kT")
            v16 = qkv.tile([P, NT, GW], F16, tag="v")  # [key-in-tile, keytile, vcol]
            ctxT = qkv.tile([P, 2, T], F32R, tag="ctxT")

            # ---------- phase 1: projections ----------
            with tc.tile_pool(name="stage", bufs=2) as stage, \
                 tc.tile_pool(name="staged_r", bufs=2) as staged_r:
                for nm, dram in (("wq", qt_d), ("wk", kt_d), ("wv", vt_d)):
                    xf = stage.tile([P, 4, T], F32, tag="xstage")
                    nc.sync.dma_start(xf[:], dram.rearrange("(ko p) t -> p ko t", p=P))
                    xr = staged_r.tile([P, 4, T], F32R, tag="xr")
                    nc.vector.tensor_copy(xr[:], xf[:])
                    w = w_r[nm]
                    if nm in ("wq", "wk"):
                        dst = qT if nm == "wq" else kT
                        bias = bq if nm == "wq" else bk
                        for pair in range(2):
                            for c in range(4):
                                ps = ps512.tile([P, D], F32, tag="mm")
                                for ko in range(4):
                                    nc.tensor.matmul(
                                        ps[:],
                                        w[:, ko, pair * P:(pair + 1) * P],
                                        xr[:, ko, c * D:(c + 1) * D],
                                        start=(ko == 0), stop=(ko == 3))
                                nc.scalar.activation(
                                    dst[:, pair, c * D:(c + 1) * D], ps[:],
                                    IDENT, bias=bias[:, pair:pair + 1])
                    else:
                        for kt_i in range(NT):
                            ps = ps512.tile([P, D], F32, tag="mm")
                            for ko in range(4):
                                nc.tensor.matmul(
                                    ps[:, :GW],
                                    xr[:, ko, kt_i * P:(kt_i + 1) * P],
                                    w[:, ko, :],
                                    start=(ko == 0), stop=(ko == 3))
                            nc.vector.tensor_copy(v16[:, kt_i, :], ps[:, :GW])

            # ---------- phase 2: attention ----------
            with tc.tile_pool(name="expT", bufs=2) as expT_pool, \
                 tc.tile_pool(name="soft", bufs=2) as soft, \
                 tc.tile_pool(name="attn", bufs=4) as attn_pool, \
                 tc.tile_pool(name="sums", bufs=4) as sums, \
                 tc.tile_pool(name="outp", bufs=2) as outp:
                NH2 = NT // 2

                # Pending attn@V work from the previous (head, half): its 32
                # matmuls are interleaved 4-per-tile into the next half's loop
                # so the PE never runs a long AV burst that starves ScalarE.
                pending = None  # (expT, h, half)

                def av_step(ii):
                    ph = pending[1]
                    ps_, pe_t = pending[3], pending[0]
                    for ci in range(2):
                        for j in (2 * ii, 2 * ii + 1):
                            nc.tensor.matmul(
                                ps_[ci][:DH, :],
                                v16[:, j, ph * DH:(ph + 1) * DH],
                                pe_t[:, j, ci * D:(ci + 1) * D],
                                start=(j == 0), stop=(j == NT - 1))

                def av_finish():
                    ph, phalf = pending[1], pending[2]
                    ppair, ps = ph // 2, ph % 2
                    for ci, c in enumerate((2 * phalf, 2 * phalf + 1)):
                        nc.vector.tensor_copy(
                            ctxT[ps * DH:(ps + 1) * DH, ppair,
                                 c * D:(c + 1) * D],
                            pending[3][ci][:DH, :])

                for h in range(GH):
                    pair, s = h // 2, h % 2
                    hq = qT[s * DH:(s + 1) * DH]
                    hk = kT[s * DH:(s + 1) * DH]
                    for half in range(2):
                        # attn^T tiles: [k-in-tile, ktile j, q-of-half]
                        expT = expT_pool.tile([P, NT, NH2 * P], F16, tag="expT")
                        for ii in range(NH2):
                            i = half * NH2 + ii
                            exp_t = soft.tile([P, T], F32, tag="exp")
                            ssum_a = sums.tile([P, 1], F32, tag="ssuma")
                            ssum_b = sums.tile([P, 1], F32, tag="ssumb")
                            for hf, acc in ((0, ssum_a), (1, ssum_b)):
                                s_ps = scores_ps.tile([P, T // 2], F32, tag="s")
                                for c in range(2):
                                    cc = 2 * hf + c
                                    nc.tensor.matmul(
                                        s_ps[:, c * D:(c + 1) * D],
                                        hq[:, pair, i * P:(i + 1) * P],
                                        hk[:, pair, cc * D:(cc + 1) * D],
                                        start=True, stop=True)
                                nc.scalar.activation(
                                    exp_t[:, hf * (T // 2):(hf + 1) * (T // 2)],
                                    s_ps[:], EXP, scale=0.125, accum_out=acc[:])
                            if pending is not None:
                                av_step(ii)
                            ssum = sums.tile([P, 1], F32, tag="ssum")
                            nc.scalar.activation(ssum[:], ssum_a[:], IDENT,
                                                 bias=ssum_b[:])
                            recip = sums.tile([P, 1], F32, tag="recip")
                            nc.vector.reciprocal(recip[:], ssum[:])
                            attn_t = attn_pool.tile([P, T], F16, tag="attn")
                            nc.vector.tensor_scalar_mul(attn_t[:], exp_t[:],
                                                        recip[:])
                            # fp32 attn output via casting SWDGE DMA
                            nc.gpsimd.dma_start(attn_d[h, i * P:(i + 1) * P, :],
                                                attn_t[:])
                            # attn^T via PE: regular fp16 matmuls against a
                            # [128, 256] doubled identity (FWL weight loads,
                            # counts as PE-busy so the HAM clock stays warm)
                            for grp in range(4):
                                t_ps = trg_ps.tile([P, 4, 2 * P], F32, tag="tr")
                                for k in range(4):
                                    j = grp * 4 + k
                                    nc.tensor.matmul(
                                        t_ps[:, k, :],
                                        attn_t[:, j * P:(j + 1) * P],
                                        eye16[:], start=True, stop=True)
                                nc.vector.tensor_copy(
                                    expT[:, grp * 4:grp * 4 + 4,
                                         ii * P:(ii + 1) * P],
                                    t_ps[:, :, :P])
                        if pending is not None:
                            av_finish()
                        av_ps_pair = [
                            ps512.tile([P, D], F32, tag="mm",
                                       name=f"av_{h}_{half}_{k}")
                            for k in range(2)]
                        pending = (expT, h, half, av_ps_pair)

                # drain the last half's attn@V
                ph, phalf = pending[1], pending[2]
                for ii in range(NH2):
                    av_step(ii)
                av_finish()

                # ---------- phase 3: out projection (partial) ----------
                for m in range(NT):
                    o_ps = ps512.tile([P, D], F32, tag="mm")
                    for pair in range(2):
                        nc.tensor.matmul(o_ps[:], ctxT[:, pair, m * P:(m + 1) * P],
                                         wo[:, pair, :],
                                         start=(pair == 0), stop=(pair == 1))
                    o_sb = outp.tile([P, D], F32, tag="o")
                    nc.scalar.copy(o_sb[:], o_ps[:])
                    nc.sync.dma_start(out_d[m * P:(m + 1) * P, :], o_sb[:])

    return nc


_program = None


def _get_program():
    global _program
    if _program is None:
        _install_wait_split()
        _program = _build_program()
    return _program


def _in_maps(Q, K, V, W_q, b_q, W_k, b_k, W_v, b_v, W_o, b_o):
    maps = []
    for b in range(4):
        qt = np.ascontiguousarray(np.asarray(Q)[b].T, dtype=np.float32)
        kt = np.ascontiguousarray(np.asarray(K)[b].T, dtype=np.float32)
        vt = np.ascontiguousarray(np.asarray(V)[b].T, dtype=np.float32)
        for g in range(2):
            sl = slice(g * GW, (g + 1) * GW)
            maps.append({
                "qt": qt, "kt": kt, "vt": vt,
                "wq": np.ascontiguousarray(np.asarray(W_q)[sl, :].T, np.float32),
                "wk": np.ascontiguousarray(np.asarray(W_k)[sl, :].T, np.float32),
                "wv": np.ascontiguousarray(np.asarray(W_v)[sl, :].T, np.float32),
                "wo": np.ascontiguousarray(np.asarray(W_o)[:, sl].T, np.float32),
                "bq": np.ascontiguousarray(
                    np.asarray(b_q)[sl].reshape(2, P).T, np.float32),
                "bk": np.ascontiguousarray(
                    np.asarray(b_k)[sl].reshape(2, P).T, np.float32),
                "eye": np.tile(np.eye(P, dtype=np.float32), (1, 2)),
            })
    return maps


def kernel(Q, K, V, W_q, b_q, W_k, b_k, W_v, b_v, W_o, b_o, _results=None):
    nc = _get_program()
    maps = _in_maps(Q, K, V, W_q, b_q, W_k, b_k, W_v, b_v, W_o, b_o)
    if _results is None:
        _results = run_bass_kernel_spmd(nc, maps, core_ids=list(range(8))).results

    attn = np.empty((4, NH, T, T), np.float32)
    out = np.empty((4, T, D), np.float32)
    # b_v passes through softmax (rows sum to 1) and b_o is additive: both
    # fold into one host-side row vector.
    extra = (np.asarray(b_v, np.float64) @ np.asarray(W_o, np.float64).T
             + np.asarray(b_o, np.float64)).astype(np.float32)
    for b in range(4):
        for g in range(2):
            r = _results[b * 2 + g]
            attn[b, g * GH:(g + 1) * GH] = r["attn_out"]
        out[b] = (_results[b * 2]["out_part"] + _results[b * 2 + 1]["out_part"]
                  + extra)
    return out, attn


# revision 25
# speedup vs baseline: 1.4473x; 1.4473x over previous
"""Multi-head attention (B=4, T=2048, D=512, 8 heads) on 8 Trainium2 cores.

Sharding: core = (batch b, head-group g) with 4 heads per group —
8 cores = 4 batches x 2 groups.  Each core computes its group's
projections, scores+softmax (attn output), attn@V and the partial
out-projection; the host sums the two group-partials per batch and
adds the biases that commute through softmax (b_v, b_o).

All matmuls run as float32r (tf32-like, full PE rate at N>=256);
softmax runs in fp32 on the Scalar engine with a fused row-sum; the
attn@V input is transposed on-chip with PE transpose-mode.
"""
import numpy as np
import concourse.bass as bass
import concourse.tile as tile
from concourse import mybir
from concourse.bass_utils import run_bass_kernel_spmd

P = 128
T = 2048
D = 512
NH = 8
DH = 64
GH = 4           # heads per group (per core)
GW = GH * DH     # 256 group width
NT = T // P      # 16 seq tiles
F32 = mybir.dt.float32
F32R = mybir.dt.float32r
EXP = mybir.ActivationFunctionType.Exp
IDENT = mybir.ActivationFunctionType.Identity


# --- walrus in this container rejects >1 sync wait per instruction; move the
# extras onto same-engine NoOps inserted just before (same semantics). ---
_patched = False


def _install_wait_split():
    global _patched
    if _patched:
        return
    _patched = True
    from concourse import bass2jax, bass_utils

    orig = bass_utils.compile_bir_kernel

    def split(bir):
        n = 0
        for fn in bir.get("functions", []):
            for blk in fn.get("blocks", []):
                out = []
                for inst in blk.get("instructions", []):
                    si = inst.get("sync_info")
                    waits = (si or {}).get("on_wait") or []
                    if len(waits) > 1:
                        for k, w in enumerate(waits[:-1]):
                            out.append({
                                "debug": inst.get("debug", 0),
                                "engine": inst["engine"],
                                "ins": [], "outs": [],
                                "name": f"{inst['name']}_wsplit{k}",
                                "opcode": "NoOp",
                                "text_hint": "waitsplit",
                                "sync_info": {"on_wait": [w], "on_update": []},
                            })
                            n += 1
                        si["on_wait"] = [waits[-1]]
                    out.append(inst)
                blk["instructions"] = out
        return n

    def patched(bir_json, tmpdir, neff_name="file.neff"):
        import json as _j
        bir = _j.loads(bir_json)
        if split(bir):
            bir_json = _j.dumps(bir).encode()
        return orig(bir_json, tmpdir, neff_name)

    bass_utils.compile_bir_kernel = patched
    bass2jax.compile_bir_kernel = patched


def _build_program():
    nc = bass.Bass()

    qt_d = nc.dram_tensor("qt", [D, T], F32, kind="ExternalInput")
    kt_d = nc.dram_tensor("kt", [D, T], F32, kind="ExternalInput")
    vt_d = nc.dram_tensor("vt", [D, T], F32, kind="ExternalInput")
    wq_d = nc.dram_tensor("wq", [D, GW], F32, kind="ExternalInput")
    wk_d = nc.dram_tensor("wk", [D, GW], F32, kind="ExternalInput")
    wv_d = nc.dram_tensor("wv", [D, GW], F32, kind="ExternalInput")
    wo_d = nc.dram_tensor("wo", [GW, D], F32, kind="ExternalInput")
    bq_d = nc.dram_tensor("bq", [P, 2], F32, kind="ExternalInput")
    bk_d = nc.dram_tensor("bk", [P, 2], F32, kind="ExternalInput")
    eye_d = nc.dram_tensor("eye", [P, P], F32, kind="ExternalInput")
    attn_d = nc.dram_tensor("attn_out", [GH, T, T], F32, kind="ExternalOutput")
    out_d = nc.dram_tensor("out_part", [T, D], F32, kind="ExternalOutput")

    with tile.TileContext(nc) as tc:
        with tc.tile_pool(name="const", bufs=1) as const, \
             tc.tile_pool(name="qkv", bufs=1) as qkv, \
             tc.tile_pool(name="scores", bufs=1, space="PSUM") as scores_ps, \
             tc.tile_pool(name="ps512", bufs=2, space="PSUM") as ps512:

            # ---------- constants ----------
            eye_f = const.tile([P, P], F32, tag="eyef")
            nc.sync.dma_start(eye_f[:], eye_d[:])
            eye = const.tile([P, P], F32R, tag="eye")
            nc.vector.tensor_copy(eye[:], eye_f[:])

            bq = const.tile([P, 2], F32, tag="bq")
            bk = const.tile([P, 2], F32, tag="bk")
            nc.sync.dma_start(bq[:], bq_d[:])
            nc.sync.dma_start(bk[:], bk_d[:])

            w_r = {}
            for nm, dram in (("wq", wq_d), ("wk", wk_d), ("wv", wv_d)):
                wf = const.tile([P, 4, GW], F32, tag="wstage")
                nc.sync.dma_start(wf[:], dram.rearrange("(ko p) m -> p ko m", p=P))
                wr = const.tile([P, 4, GW], F32R, tag=f"{nm}r")
                nc.vector.tensor_copy(wr[:], wf[:])
                w_r[nm] = wr
            wof = const.tile([P, 2, D], F32, tag="wof")
            nc.sync.dma_start(wof[:], wo_d.rearrange("(pp p) m -> p pp m", p=P))
            wo = const.tile([P, 2, D], F32R, tag="wo")
            nc.vector.tensor_copy(wo[:], wof[:])

            # ---------- persistent activations ----------
            qT = qkv.tile([P, 2, T], F32R, tag="qT")    # [dh-of-pair, pair, seq]
            kT = qkv.tile([P, 2, T], F32R, tag="kT")
            v_sb = qkv.tile([P, NT, GW], F32R, tag="v")  # [key-in-tile, keytile, vcol]
            ctxT = qkv.tile([P, 2, T], F32R, tag="ctxT")

            # ---------- phase 1: projections ----------
            with tc.tile_pool(name="stage", bufs=2) as stage, \
                 tc.tile_pool(name="staged_r", bufs=1) as staged_r:
                for nm, dram in (("wq", qt_d), ("wk", kt_d), ("wv", vt_d)):
                    xf = stage.tile([P, 4, T], F32, tag="xstage")
                    nc.sync.dma_start(xf[:], dram.rearrange("(ko p) t -> p ko t", p=P))
                    xr = staged_r.tile([P, 4, T], F32R, tag="xr")
                    nc.vector.tensor_copy(xr[:], xf[:])
                    w = w_r[nm]
                    if nm in ("wq", "wk"):
                        dst = qT if nm == "wq" else kT
                        bias = bq if nm == "wq" else bk
                        for pair in range(2):
                            for c in range(4):
                                ps = ps512.tile([P, D], F32, tag="mm")
                                for ko in range(4):
                                    nc.tensor.matmul(
                                        ps[:],
                                        w[:, ko, pair * P:(pair + 1) * P],
                                        xr[:, ko, c * D:(c + 1) * D],
                                        start=(ko == 0), stop=(ko == 3))
                                nc.scalar.activation(
                                    dst[:, pair, c * D:(c + 1) * D], ps[:],
                                    IDENT, bias=bias[:, pair:pair + 1])
                    else:
                        for kt_i in range(NT):
                            ps = ps512.tile([P, D], F32, tag="mm")
                            for ko in range(4):
                                nc.tensor.matmul(
                                    ps[:, :GW],
                                    xr[:, ko, kt_i * P:(kt_i + 1) * P],
                                    w[:, ko, :],
                                    start=(ko == 0), stop=(ko == 3))
                            nc.vector.tensor_copy(v_sb[:, kt_i, :], ps[:, :GW])

            # ---------- phase 2: attention ----------
            with tc.tile_pool(name="expT", bufs=1) as expT_pool, \
                 tc.tile_pool(name="soft", bufs=2) as soft, \
                 tc.tile_pool(name="attn", bufs=3) as attn_pool, \
                 tc.tile_pool(name="sums", bufs=4) as sums, \
                 tc.tile_pool(name="outp", bufs=2) as outp:
                HQ = T // 2   # half of the query range handled per expT fill
                for h in range(GH):
                    pair, s = h // 2, h % 2
                    hq = qT[s * DH:(s + 1) * DH]
                    hk = kT[s * DH:(s + 1) * DH]
                    for half in range(2):
                        expT = expT_pool.tile([P, NT, HQ], F32R, tag="expT")
                        for ii in range(NT // 2):
                            i = half * (NT // 2) + ii
                            s_ps = scores_ps.tile([P, T], F32, tag="s")
                            for c in range(4):
                                nc.tensor.matmul(
                                    s_ps[:, c * D:(c + 1) * D],
                                    hq[:, pair, i * P:(i + 1) * P],
                                    hk[:, pair, c * D:(c + 1) * D],
                                    start=True, stop=True)
                            exp_t = soft.tile([P, T], F32, tag="exp")
                            ssum = sums.tile([P, 1], F32, tag="ssum")
                            nc.scalar.activation(exp_t[:], s_ps[:], EXP,
                                                 scale=0.125, accum_out=ssum[:])
                            recip = sums.tile([P, 1], F32, tag="recip")
                            nc.vector.reciprocal(recip[:], ssum[:])
                            attn_t = attn_pool.tile([P, T], F32R, tag="attn")
                            nc.vector.tensor_scalar_mul(attn_t[:], exp_t[:], recip[:])
                            nc.sync.dma_start(attn_d[h, i * P:(i + 1) * P, :],
                                              attn_t[:].bitcast(F32))
                            for grp in range(4):
                                t_ps = ps512.tile([P, D], F32R, tag="tr")
                                for k in range(4):
                                    j = grp * 4 + k
                                    nc.tensor.transpose(
                                        t_ps[:, k * P:(k + 1) * P],
                                        attn_t[:, j * P:(j + 1) * P], eye[:])
                                nc.vector.tensor_copy(
                                    expT[:, grp * 4:grp * 4 + 4, ii * P:(ii + 1) * P],
                                    t_ps[:].rearrange("p (j q) -> p j q", j=4))
                        # attn @ V for this head, this half of the queries
                        for c in range(2):
                            qc = half * 2 + c
                            av_ps = ps512.tile([P, D], F32, tag="mm")
                            for j in range(NT):
                                nc.tensor.matmul(
                                    av_ps[:DH, :],
                                    v_sb[:, j, h * DH:(h + 1) * DH],
                                    expT[:, j, c * D:(c + 1) * D],
                                    start=(j == 0), stop=(j == NT - 1))
                            nc.vector.tensor_copy(
                                ctxT[s * DH:(s + 1) * DH, pair, qc * D:(qc + 1) * D],
                                av_ps[:DH, :])

                # ---------- phase 3: out projection (partial) ----------
                for m in range(NT):
                    o_ps = ps512.tile([P, D], F32, tag="mm")
                    for pair in range(2):
                        nc.tensor.matmul(o_ps[:], ctxT[:, pair, m * P:(m + 1) * P],
                                         wo[:, pair, :],
                                         start=(pair == 0), stop=(pair == 1))
                    o_sb = outp.tile([P, D], F32, tag="o")
                    nc.scalar.copy(o_sb[:], o_ps[:])
                    nc.sync.dma_start(out_d[m * P:(m + 1) * P, :], o_sb[:])

    return nc


_program = None


def _get_program():
    global _program
    if _program is None:
        _install_wait_split()
        _program = _build_program()
    return _program


def _in_maps(Q, K, V, W_q, b_q, W_k, b_k, W_v, b_v, W_o, b_o):
    eye = np.eye(P, dtype=np.float32)
    maps = []
    for b in range(4):
        qt = np.ascontiguousarray(np.asarray(Q)[b].T, dtype=np.float32)
        kt = np.ascontiguousarray(np.asarray(K)[b].T, dtype=np.float32)
        vt = np.ascontiguousarray(np.asarray(V)[b].T, dtype=np.float32)
        for g in range(2):
            sl = slice(g * GW, (g + 1) * GW)
            maps.append({
                "qt": qt, "kt": kt, "vt": vt,
                "wq": np.ascontiguousarray(np.asarray(W_q)[sl, :].T, np.float32),
                "wk": np.ascontiguousarray(np.asarray(W_k)[sl, :].T, np.float32),
                "wv": np.ascontiguousarray(np.asarray(W_v)[sl, :].T, np.float32),
                "wo": np.ascontiguousarray(np.asarray(W_o)[:, sl].T, np.float32),
                "bq": np.ascontiguousarray(
                    np.asarray(b_q)[sl].reshape(2, P).T, np.float32),
                "bk": np.ascontiguousarray(
                    np.asarray(b_k)[sl].reshape(2, P).T, np.float32),
                "eye": eye,
            })
    return maps


def kernel(Q, K, V, W_q, b_q, W_k, b_k, W_v, b_v, W_o, b_o, _results=None):
    nc = _get_program()
    maps = _in_maps(Q, K, V, W_q, b_q, W_k, b_k, W_v, b_v, W_o, b_o)
    if _results is None:
        _results = run_bass_kernel_spmd(nc, maps, core_ids=list(range(8))).results

    attn = np.empty((4, NH, T, T), np.float32)
    out = np.empty((4, T, D), np.float32)
    # b_v passes through softmax (rows sum to 1) and b_o is additive: both
    # fold into one host-side row vector.
    extra = (np.asarray(b_v, np.float64) @ np.asarray(W_o, np.float64).T
             + np.asarray(b_o, np.float64)).astype(np.float32)
    for b in range(4):
        for g in range(2):
            r = _results[b * 2 + g]
            attn[b, g * GH:(g + 1) * GH] = r["attn_out"]
        out[b] = (_results[b * 2]["out_part"] + _results[b * 2 + 1]["out_part"]
                  + extra)
    return out, attn
